# Initial kernel scaffold
#
"""Multi-head self-attention (causal) Trainium2 Bass kernel, 8-core SPMD.

Sharding: 8 cores = 2 batches x 4 head-groups (3 heads each).
Each core computes, for its (batch, head-group):
  - Q^T, K^T, V projections from a host-pretransposed x^T (bf16)
  - causal attention with scores kept transposed (S^T[k,q]) so no on-device
    transposes are needed; softmax denominator comes free via a ones-column
    appended to V
  - its 3 heads' slice of the output projection (partial sum over d)
Host gathers: out[b] = sum of 4 group partials + (b_proj + b_v @ W_proj).
b_k is dropped (softmax row-shift invariance), b_v folded into host bias.

Layout notes: heads 0/1 are packed into partition halves 0:64 / 64:128 so
their score matmuls land in different PE row groups (concurrent on HW) and
the projection contracts over 128 partitions in one matmul. Cross-partition
moves (head-2 Q^T, head-1 attn^T) are done with small SBUF->SBUF DMAs,
which are the only engines that can re-partition.
"""

import numpy as np
import ml_dtypes

S = 2048          # sequence length
D = 768           # model dim
HD = 64           # head dim
HPC = 3           # heads per core
NCORES = 8
P = 128           # partitions
CT = D // P       # 6 contraction tiles over model dim
KT = S // P       # 16 key tiles
QC = 512          # query chunk (PSUM bank width in fp32)
NQC = S // QC     # 4 query chunks

_BF = ml_dtypes.bfloat16

_cache = {}


def _build_nc():
    import concourse.bass as bass
    import concourse.mybir as mybir
    import concourse.tile as tile
    from concourse import bacc
    from contextlib import ExitStack

    bf = mybir.dt.bfloat16
    f32 = mybir.dt.float32

    nc = bacc.Bacc()
    xT = nc.declare_dram_parameter("xT", [D, S], bf, isOutput=False)
    # 3 lhsT slots per c-tile: 0=[Wk0|Wk1] 1=[Wq0|Wq1] 2=[Wk2|Wq2]
    w_qk = nc.declare_dram_parameter("w_qk", [D, 3, P], bf, isOutput=False)
    w_v = nc.declare_dram_parameter("w_v", [D, HPC * HD], bf, isOutput=False)
    # col 0: [bq_h0 | bq_h1]; col 1: rows 64:128 = bq_h2
    bq = nc.declare_dram_parameter("bq", [P, 2], f32, isOutput=False)
    # rows: W_proj rows of h0, h1, h2 stacked
    w_p = nc.declare_dram_parameter("w_p", [HPC * HD, D], bf, isOutput=False)
    mask = nc.declare_dram_parameter("mask", [P, P], bf, isOutput=False)
    out_p = nc.declare_dram_parameter("out_p", [S, D], f32, isOutput=True)

    Exp = mybir.ActivationFunctionType.Exp

    with tile.TileContext(nc) as tc, ExitStack() as ctx:
        singles = ctx.enter_context(tc.tile_pool(name="singles", bufs=1))
        pmm = ctx.enter_context(tc.tile_pool(name="pmm", bufs=2, space="PSUM"))
        # scores + projection share this pool's 3 banks (disjoint phases)
        ps_pool = ctx.enter_context(tc.tile_pool(name="ps", bufs=3, space="PSUM"))
        po_pool = ctx.enter_context(tc.tile_pool(name="po", bufs=3, space="PSUM"))
        pt_pool = ctx.enter_context(tc.tile_pool(name="pt", bufs=12))
        norm_pool = ctx.enter_context(tc.tile_pool(name="norm", bufs=3))
        outs_pool = ctx.enter_context(tc.tile_pool(name="outs", bufs=3))

        # ---- persistent SBUF ----
        # DMA issue costs ~0.5us on the issuing engine: keep the count low,
        # small critical inputs first, and bulk loads split SP/gpsimd.
        xT_s = singles.tile([P, CT, S], bf)
        wqk_s = singles.tile([P, CT, 3, P], bf)
        wv_s = singles.tile([P, CT, HPC * HD], bf)
        bq_s = singles.tile([P, 2], f32)
        mask_s = singles.tile([P, P], bf)
        wpa_s = singles.tile([P, D], bf)
        wpb_s = singles.tile([HD, D], bf)
        wpb1_s = singles.tile([HD, D], bf)  # h1 proj rows at base partition 0
        xt_r = xT.rearrange("(t p) q -> p t q", p=P)
        nc.gpsimd.dma_start(out=xT_s[:, 0:1, 0:QC], in_=xt_r[:, 0:1, 0:QC])
        nc.gpsimd.dma_start(out=xT_s[:, 1:3, 0:QC], in_=xt_r[:, 1:3, 0:QC])
        nc.scalar.dma_start(out=xT_s[:, 3:CT, 0:QC], in_=xt_r[:, 3:CT, 0:QC])
        wqk_r = w_qk.rearrange("(t p) s m -> p t s m", p=P)
        nc.sync.dma_start(out=wqk_s[:, 0:1], in_=wqk_r[:, 0:1])
        nc.sync.dma_start(out=wqk_s[:, 1:3], in_=wqk_r[:, 1:3])
        nc.sync.dma_start(out=wqk_s[:, 3:CT], in_=wqk_r[:, 3:CT])
        nc.gpsimd.dma_start(out=bq_s, in_=bq[:])
        nc.gpsimd.dma_start(out=mask_s, in_=mask[:])
        wv_r = w_v.rearrange("(t p) m -> p t m", p=P)
        nc.gpsimd.dma_start(out=wv_s[:, 0:3], in_=wv_r[:, 0:3])
        nc.gpsimd.dma_start(out=wv_s[:, 3:CT], in_=wv_r[:, 3:CT])
        for qc in range(1, NQC):
            nc.gpsimd.dma_start(out=xT_s[:, :, qc * QC:(qc + 1) * QC],
                                in_=xt_r[:, :, qc * QC:(qc + 1) * QC])
        nc.gpsimd.dma_start(out=wpa_s, in_=w_p[0:P, :])
        nc.gpsimd.dma_start(out=wpb1_s, in_=w_p[HD:P, :])
        nc.gpsimd.dma_start(out=wpb_s, in_=w_p[P:P + HD, :])

        # Q^T/K^T: slot 0 holds head0 (parts 0:64) + head1 (parts 64:128),
        # slot 1 holds head2 (parts 0:64; qt slot1 filled via repartition DMA).
        qt_s = singles.tile([P, 2, S], bf)
        kt_s = singles.tile([P, 2, S], bf)
        # V with a ones column appended per head (softmax denominator trick)
        v_s = singles.tile([P, KT, HPC, HD + 1], bf)
        nc.gpsimd.memset(v_s[:, :, :, HD:HD + 1], 1.0)
        # attn^T: h0 at parts 0:64, h1 at parts 64:128 (via DMA), h2 separate
        attn01_s = singles.tile([P, S], bf)
        attn2_s = singles.tile([HD, S], bf)
        attn1b_s = singles.tile([HD, QC], bf)  # last chunk's h1, un-repartitioned

        def proj_fillers(c, use_act=False, three_way=False):
            # output projection of chunk c, one filler per q-tile.
            # three_way (last chunk): one matmul per head so the projection
            # starts as soon as head 0 is normalized, skipping the h1
            # repartition DMA on the critical tail.
            def one(t):
                def f(dep=None):
                    ob = outs_pool.tile([P, D], f32, tag="ob", name="ob")
                    for e0, en in ((0, 512), (512, 256)):
                        pp = pmm.tile([P, QC], f32, tag="mm", name="pp")
                        if three_way:
                            tb = t * P - (NQC - 1) * QC
                            nc.tensor.matmul(pp[:, 0:en],
                                             lhsT=attn01_s[0:HD, t * P:(t + 1) * P],
                                             rhs=wpa_s[0:HD, e0:e0 + en],
                                             start=True, stop=False)
                            nc.tensor.matmul(pp[:, 0:en],
                                             lhsT=attn1b_s[:, tb:tb + P],
                                             rhs=wpb1_s[:, e0:e0 + en],
                                             start=False, stop=False)
                            mm = None
                        else:
                            mm = nc.tensor.matmul(pp[:, 0:en],
                                                  lhsT=attn01_s[:, t * P:(t + 1) * P],
                                                  rhs=wpa_s[:, e0:e0 + en],
                                                  start=True, stop=False)
                        if dep is not None and mm is not None:
                            tile.add_dep_helper(mm.ins, dep.ins, sync=False,
                                                reason="hold filler past last AV")
                            dep = None
                        nc.tensor.matmul(pp[:, 0:en],
                                         lhsT=attn2_s[:, t * P:(t + 1) * P],
                                         rhs=wpb_s[:, e0:e0 + en],
                                         start=False, stop=True)
                        if use_act:  # tail: ACT is idle, DVE still normalizing
                            nc.scalar.copy(out=ob[:, e0:e0 + en], in_=pp[:, 0:en])
                            nc.sync.dma_start(
                                out=out_p[t * P:(t + 1) * P, e0:e0 + en],
                                in_=ob[:, e0:e0 + en])
                        else:
                            nc.vector.tensor_copy(out=ob[:, e0:e0 + en], in_=pp[:, 0:en])
                    if not use_act:
                        nc.sync.dma_start(out=out_p[t * P:(t + 1) * P, :], in_=ob)
                return f
            return [one(t) for t in range(4 * c, 4 * c + 4)]

        def qkv_fillers(c):
            # Q^T/K^T/V projections for chunk c, as 7 PE filler groups
            qs = c * QC
            qsl = slice(qs, qs + QC)

            def g_kk():
                ps_kk = pmm.tile([P, QC], f32, tag="mm", name="ps_kk")
                for ct in range(CT):
                    nc.tensor.matmul(ps_kk, lhsT=wqk_s[:, ct, 0, :],
                                     rhs=xT_s[:, ct, qsl],
                                     start=(ct == 0), stop=(ct == CT - 1))
                nc.vector.tensor_copy(out=kt_s[:, 0, qsl], in_=ps_kk)

            def g_qq():
                ps_qq = pmm.tile([P, QC], f32, tag="mm", name="ps_qq")
                for ct in range(CT):
                    nc.tensor.matmul(ps_qq, lhsT=wqk_s[:, ct, 1, :],
                                     rhs=xT_s[:, ct, qsl],
                                     start=(ct == 0), stop=(ct == CT - 1))
                nc.vector.tensor_scalar_add(out=qt_s[:, 0, qsl], in0=ps_qq,
                                            scalar1=bq_s[:, 0:1])

            def g_kq2():
                ps_kq2 = pmm.tile([P, QC], f32, tag="mm", name="ps_kq2")
                for ct in range(CT):
                    nc.tensor.matmul(ps_kq2, lhsT=wqk_s[:, ct, 2, :],
                                     rhs=xT_s[:, ct, qsl],
                                     start=(ct == 0), stop=(ct == CT - 1))
                nc.vector.tensor_copy(out=kt_s[0:HD, 1, qsl], in_=ps_kq2[0:HD, :])
                # head2 Q lands in parts 64:128; bias-add, then repartition DMA
                q2st = norm_pool.tile([P, QC], bf, tag="q2st", name="q2st")
                nc.vector.tensor_scalar_add(out=q2st[HD:P, :], in0=ps_kq2[HD:P, :],
                                            scalar1=bq_s[HD:P, 1:2])
                nc.sync.dma_start(out=qt_s[0:HD, 1, qsl], in_=q2st[HD:P, :])

            def g_v(kt):
                def f():
                    ps_v = pmm.tile([P, QC], f32, tag="mm", name="ps_v")
                    for ct in range(CT):
                        nc.tensor.matmul(ps_v[:, 0:HPC * HD],
                                         lhsT=xT_s[:, ct, kt * P:(kt + 1) * P],
                                         rhs=wv_s[:, ct, :],
                                         start=(ct == 0), stop=(ct == CT - 1))
                    nc.vector.tensor_copy(
                        out=v_s[:, kt, :, 0:HD],
                        in_=ps_v[:, 0:HPC * HD].rearrange("p (h d) -> p h d", h=HPC))
                return f
            return [g_kk, g_qq, g_kq2] + [g_v(kt) for kt in range(4 * c, 4 * c + 4)]

        hsl = [slice(0, HD), slice(HD, P), slice(0, HD)]
        hslot = [0, 0, 1]

        for f in qkv_fillers(0):
            f()

        for c in range(NQC):
            qs = c * QC
            qsl = slice(qs, qs + QC)
            # fillers woven into this chunk's attention: next chunk's QKV,
            # then the previous chunk's projection
            fillers = []
            if c + 1 < NQC:
                fillers += qkv_fillers(c + 1)
            if c > 0:
                fillers += proj_fillers(c - 1)

            # ---- attention for q-chunk c (kt-major; h0/h1 scores can overlap
            # in different PE row groups) ----
            po = [po_pool.tile([P, QC], f32, tag="po", name=f"po{h}")
                  for h in range(HPC)]
            nkt = 4 * c + 4

            def emit_scores(kt):
                off = max(0, kt * P - qs)
                n = QC - off
                pss = []
                for h in range(HPC):
                    ps_s = ps_pool.tile([P, QC], f32, tag="ss", name="ps_s")
                    nc.tensor.matmul(ps_s[:, 0:n],
                                     lhsT=kt_s[hsl[h], hslot[h], kt * P:(kt + 1) * P],
                                     rhs=qt_s[hsl[h], hslot[h], qs + off:qs + QC],
                                     start=True, stop=True)
                    pss.append(ps_s)
                return pss

            def emit_avs(kt, pss):
                off = max(0, kt * P - qs)
                n = QC - off
                diag = kt * P >= qs
                mm = None
                for h in range(HPC):
                    pt = pt_pool.tile([P, QC], bf, tag="pt", name="pt")
                    nc.scalar.activation(out=pt[:, off:QC], in_=pss[h][:, 0:n],
                                         func=Exp, scale=0.125)
                    if diag:  # mask k>q inside the diagonal 128x128 block
                        nc.gpsimd.tensor_mul(out=pt[:, off:off + P],
                                             in0=pt[:, off:off + P], in1=mask_s)
                    mm = nc.tensor.matmul(po[h][0:HD + 1, off:QC],
                                          lhsT=v_s[:, kt, h, :], rhs=pt[:, off:QC],
                                          start=(kt == 0), stop=(kt == nkt - 1))
                return mm

            emitted = 0
            # last chunk: reserve two fillers to run after the final AV (they
            # bridge the normalize window), pace the rest into the kt loop
            n_defer = 0
            n_pace = len(fillers) - n_defer
            w0 = 1 if c + 1 < NQC else max(1, nkt - 3 * len(fillers))
            prev = emit_scores(0)
            for kt in range(1, nkt):
                cur = emit_scores(kt)
                emit_avs(kt - 1, prev)
                prev = cur
                if kt >= w0:
                    want = ((kt - w0 + 1) * n_pace) // max(1, nkt - w0)
                    while emitted < want:
                        fillers[emitted]()
                        emitted += 1
            last_av = emit_avs(nkt - 1, prev)
            while emitted < n_pace:
                fillers[emitted]()
                emitted += 1
            while emitted < len(fillers):
                fillers[emitted](dep=last_av)
                emitted += 1

            # normalize: recip of denom (DVE), broadcast (Pool), multiply (DVE)
            for h in range(HPC):
                recip = norm_pool.tile([HD + 1, QC], f32, tag="recip", name="recip")
                nc.vector.reciprocal(out=recip[HD:HD + 1, :], in_=po[h][HD:HD + 1, :])
                bcast = norm_pool.tile([HD, 1, QC], f32, tag="bcast", name="bcast")
                rs = recip[HD:HD + 1, :]
                rep = bass.AP(tensor=rs.tensor, offset=rs.offset,
                              ap=[list(rs.ap[0]), [0, HD], list(rs.ap[1])])
                nc.gpsimd.dma_start(out=bcast, in_=rep)
                if h == 0:
                    nc.vector.tensor_mul(out=attn01_s[0:HD, qsl], in0=po[h][0:HD, :],
                                         in1=bcast[:, 0, :])
                elif h == 1:
                    if c + 1 == NQC:
                        nc.vector.tensor_mul(out=attn1b_s, in0=po[h][0:HD, :],
                                             in1=bcast[:, 0, :])
                    else:
                        a1 = norm_pool.tile([HD, QC], bf, tag="a1", name="a1")
                        nc.vector.tensor_mul(out=a1, in0=po[h][0:HD, :],
                                             in1=bcast[:, 0, :])
                        nc.sync.dma_start(out=attn01_s[HD:P, qsl], in_=a1)
                else:
                    nc.vector.tensor_mul(out=attn2_s[:, qsl], in0=po[h][0:HD, :],
                                         in1=bcast[:, 0, :])

        for f in proj_fillers(NQC - 1, use_act=True, three_way=True):
            f()

    nc.compile()
    return nc


def _prep_inputs(x, W_qkv, b_qkv, W_proj):
    """Build the 8 per-core input maps (all bf16 except biases)."""
    in_maps = []
    for cid in range(NCORES):
        b, g = divmod(cid, 4)
        hs = [g * HPC + i for i in range(HPC)]  # global head ids

        def wslice(kind, h):  # kind 0=q 1=k 2=v
            return W_qkv[:, kind * D + h * HD:(kind * D + (h + 1) * HD)]

        xT = np.ascontiguousarray(x[b].T).astype(_BF)

        w_qk = np.zeros((D, 3, P), dtype=np.float32)
        w_qk[:, 0, 0:HD] = wslice(1, hs[0])
        w_qk[:, 0, HD:P] = wslice(1, hs[1])
        w_qk[:, 1, 0:HD] = wslice(0, hs[0])
        w_qk[:, 1, HD:P] = wslice(0, hs[1])
        w_qk[:, 2, 0:HD] = wslice(1, hs[2])
        w_qk[:, 2, HD:P] = wslice(0, hs[2])

        w_v = np.concatenate([wslice(2, h) for h in hs], axis=1)

        bq = np.zeros((P, 2), dtype=np.float32)
        bq[0:HD, 0] = b_qkv[hs[0] * HD:(hs[0] + 1) * HD]
        bq[HD:P, 0] = b_qkv[hs[1] * HD:(hs[1] + 1) * HD]
        bq[HD:P, 1] = b_qkv[hs[2] * HD:(hs[2] + 1) * HD]

        w_p = np.concatenate([W_proj[h * HD:(h + 1) * HD, :] for h in hs], axis=0)

        mask = np.triu(np.ones((P, P), dtype=np.float32))

        in_maps.append({
            "xT": xT,
            "w_qk": w_qk.astype(_BF),
            "w_v": w_v.astype(_BF),
            "bq": bq,
            "w_p": w_p.astype(_BF),
            "mask": mask.astype(_BF),
        })
    return in_maps


def _run(inputs, trace=False):
    from concourse.bass_utils import run_bass_kernel_spmd

    x = np.asarray(inputs["x"], dtype=np.float32)
    W_qkv = np.asarray(inputs["W_qkv"], dtype=np.float32)
    b_qkv = np.asarray(inputs["b_qkv"], dtype=np.float32)
    W_proj = np.asarray(inputs["W_proj"], dtype=np.float32)
    b_proj = np.asarray(inputs["b_proj"], dtype=np.float32)

    if "nc" not in _cache:
        _cache["nc"] = _build_nc()
    nc = _cache["nc"]

    in_maps = _prep_inputs(x, W_qkv, b_qkv, W_proj)
    res = run_bass_kernel_spmd(nc, in_maps, core_ids=list(range(NCORES)),
                               trace=trace)

    host_bias = b_proj + b_qkv[2 * D:3 * D] @ W_proj  # b_v folded through proj
    B = x.shape[0]
    out = np.zeros((B, S, D), dtype=np.float32)
    for cid in range(NCORES):
        b = cid // 4
        out[b] += res.results[cid]["out_p"]
    out += host_bias
    return out, res


def kernel(x, W_qkv, b_qkv, W_proj, b_proj):
    out, _ = _run({"x": x, "W_qkv": W_qkv, "b_qkv": b_qkv,
                   "W_proj": W_proj, "b_proj": b_proj})
    return out



# revision 2
# speedup vs baseline: 1.0208x; 1.0208x over previous
"""Multi-head self-attention (causal) Trainium2 Bass kernel, 8-core SPMD. v2.

Sharding: 8 cores = 2 batches x 4 head-groups (3 heads each), as v1.

v2 restructure vs v1 (cost model charges matmuls by output free-dim columns):
  - AV computed in STRAIGHT form out[q,d], kt-major: after each kt's exp,
    every live (q-tile, head) chain of the chunk accumulates that kt's
    65-col contribution (64 d + ones-denominator). Halves AV's PE columns
    vs the transposed form and spreads them evenly across the exp stream.
    The 12 concurrent chains share one 2-bank PSUM tile, zeroed by a memset
    and accumulated with start=False (PSUM auto-zero is 2KB-granular, so
    independent start=True chains cannot share banks).
  - Per-q-tile output normalized via per-partition scalar recip, then
    PE-transposed (identity rhs) to [d, q] for the projection; h0/h1 share
    one [128,128] transpose.
  - softmax exp batched 3 heads per ACT instruction ([128,3,256] PSUM score
    tiles) - 1/3 the ACT access-latency overhead.
  - output partials stored bf16 (half the write DMA traffic).
  - deficit-paced PE fill queue: next-chunk QKV (K/V per-ktile late-split
    for the last chunk) and per-tile transpose+projection tails are
    deferred into ACT-paced slots so PE never starves.

PSUM budget (8 banks): scores [128,3,256]x2 = 4, AV accum = 2, pmm x2 = 2.
"""

import numpy as np
import ml_dtypes

S = 2048          # sequence length
D = 768           # model dim
HD = 64           # head dim
HPC = 3           # heads per core
NCORES = 8
P = 128           # partitions
CT = D // P       # 6 contraction tiles over model dim
KT = S // P       # 16 key tiles
QC = 512          # query chunk
NQC = S // QC     # 4 query chunks
HF = 256          # score-psum half-chunk (2 banks for [128,3,HF] f32)

_BF = ml_dtypes.bfloat16

_cache = {}


def _build_nc():
    import concourse.bass as bass
    import concourse.mybir as mybir
    import concourse.tile as tile
    from concourse import bacc
    from contextlib import ExitStack
    from collections import deque

    bf = mybir.dt.bfloat16
    f32 = mybir.dt.float32

    nc = bacc.Bacc()
    xT = nc.declare_dram_parameter("xT", [D, S], bf, isOutput=False)
    # 3 lhsT slots per c-tile: 0=[Wk0|Wk1] 1=[Wq0|Wq1] 2=[Wk2|Wq2]
    w_qk = nc.declare_dram_parameter("w_qk", [D, 3, P], bf, isOutput=False)
    w_v = nc.declare_dram_parameter("w_v", [D, HPC * HD], bf, isOutput=False)
    # col 0: [bq_h0 | bq_h1]; col 1: rows 64:128 = bq_h2
    bq = nc.declare_dram_parameter("bq", [P, 2], f32, isOutput=False)
    # rows: W_proj rows of h0, h1, h2 stacked
    w_p = nc.declare_dram_parameter("w_p", [HPC * HD, D], bf, isOutput=False)
    mask = nc.declare_dram_parameter("mask", [P, P], bf, isOutput=False)
    ident = nc.declare_dram_parameter("ident", [P, P], bf, isOutput=False)
    out_p = nc.declare_dram_parameter("out_p", [S, D], bf, isOutput=True)

    Exp = mybir.ActivationFunctionType.Exp

    with tile.TileContext(nc) as tc, ExitStack() as ctx:
        singles = ctx.enter_context(tc.tile_pool(name="singles", bufs=1))
        ps3 = ctx.enter_context(tc.tile_pool(name="ps3", bufs=2, space="PSUM"))
        pav = ctx.enter_context(tc.tile_pool(name="pav", bufs=1, space="PSUM"))
        pmm = ctx.enter_context(tc.tile_pool(name="pmm", bufs=2, space="PSUM"))
        ptp = ctx.enter_context(tc.tile_pool(name="ptp", bufs=18))
        np_pool = ctx.enter_context(tc.tile_pool(name="npool", bufs=3))
        ao_pool = ctx.enter_context(tc.tile_pool(name="aop", bufs=8))
        at_pool = ctx.enter_context(tc.tile_pool(name="atp", bufs=8))
        outs_pool = ctx.enter_context(tc.tile_pool(name="outs", bufs=3))

        # ---- persistent SBUF ----
        xT_s = singles.tile([P, CT, S], bf)
        wqk_s = singles.tile([P, CT, 3, P], bf)
        wv_s = singles.tile([P, CT, HPC * HD], bf)
        bq_s = singles.tile([P, 2], f32)
        mask_s = singles.tile([P, P], bf)
        ident_s = singles.tile([P, P], bf)
        wpa_s = singles.tile([P, D], bf)
        wpb_s = singles.tile([HD, D], bf)
        qt_s = singles.tile([P, 2, S], bf)
        kt_s = singles.tile([P, 2, S], bf)
        v_s = singles.tile([P, KT, HPC, HD + 1], bf)

        xt_r = xT.rearrange("(t p) q -> p t q", p=P)
        wqk_r = w_qk.rearrange("(t p) s m -> p t s m", p=P)
        wv_r = w_v.rearrange("(t p) m -> p t m", p=P)
        # critical-path loads first: chunk-0 QKV needs wqk + xT cols 0:512,
        # finely split so the first matmuls can start ASAP
        nc.sync.dma_start(out=wqk_s[:, 0:1], in_=wqk_r[:, 0:1])
        nc.scalar.dma_start(out=xT_s[:, 0:3, 0:HF], in_=xt_r[:, 0:3, 0:HF])
        nc.sync.dma_start(out=wqk_s[:, 1:3], in_=wqk_r[:, 1:3])
        nc.scalar.dma_start(out=xT_s[:, 3:CT, 0:HF], in_=xt_r[:, 3:CT, 0:HF])
        nc.sync.dma_start(out=wqk_s[:, 3:CT], in_=wqk_r[:, 3:CT])
        nc.scalar.dma_start(out=xT_s[:, 0:3, HF:QC], in_=xt_r[:, 0:3, HF:QC])
        nc.scalar.dma_start(out=xT_s[:, 3:CT, HF:QC], in_=xt_r[:, 3:CT, HF:QC])
        nc.gpsimd.dma_start(out=bq_s, in_=bq[:])
        nc.gpsimd.dma_start(out=mask_s, in_=mask[:])
        nc.gpsimd.dma_start(out=ident_s, in_=ident[:])
        nc.gpsimd.dma_start(out=wv_s[:, 0:3], in_=wv_r[:, 0:3])
        nc.gpsimd.dma_start(out=wv_s[:, 3:CT], in_=wv_r[:, 3:CT])
        for qc in range(1, NQC):
            nc.gpsimd.dma_start(out=xT_s[:, :, qc * QC:(qc + 1) * QC],
                                in_=xt_r[:, :, qc * QC:(qc + 1) * QC])
        nc.gpsimd.dma_start(out=wpa_s, in_=w_p[0:P, :])
        nc.gpsimd.dma_start(out=wpb_s, in_=w_p[P:P + HD, :])
        nc.gpsimd.memset(v_s[:, :, :, HD:HD + 1], 1.0)

        # head slices in the packed Q^T/K^T layout
        hsl = [slice(0, HD), slice(HD, P), slice(0, HD)]
        hslot = [0, 0, 1]

        # ---- QKV projection work items: (est_cols, fn) ----
        def g_kk(c, t0, t1):
            # K^T (heads 0/1) for k-tiles t0:t1
            lo, hi = t0 * P, t1 * P

            def f():
                ps_kk = pmm.tile([P, QC], f32, tag="mm", name="ps_kk")
                n = hi - lo
                for ct in range(CT):
                    nc.tensor.matmul(ps_kk[:, 0:n], lhsT=wqk_s[:, ct, 0, :],
                                     rhs=xT_s[:, ct, lo:hi],
                                     start=(ct == 0), stop=(ct == CT - 1))
                nc.vector.tensor_copy(out=kt_s[:, 0, lo:hi], in_=ps_kk[:, 0:n])
            return (6 * (hi - lo), f)

        def g_qq(c, half):
            qsl = slice(c * QC + half * HF, c * QC + (half + 1) * HF)

            def f():
                ps_qq = pmm.tile([P, QC], f32, tag="mm", name="ps_qq")
                for ct in range(CT):
                    nc.tensor.matmul(ps_qq[:, 0:HF], lhsT=wqk_s[:, ct, 1, :],
                                     rhs=xT_s[:, ct, qsl],
                                     start=(ct == 0), stop=(ct == CT - 1))
                nc.vector.tensor_scalar_add(out=qt_s[:, 0, qsl],
                                            in0=ps_qq[:, 0:HF],
                                            scalar1=bq_s[:, 0:1])
            return (6 * HF, f)

        def g_kq2(c, half):
            qsl = slice(c * QC + half * HF, c * QC + (half + 1) * HF)

            def f():
                ps_kq2 = pmm.tile([P, QC], f32, tag="mm", name="ps_kq2")
                for ct in range(CT):
                    nc.tensor.matmul(ps_kq2[:, 0:HF], lhsT=wqk_s[:, ct, 2, :],
                                     rhs=xT_s[:, ct, qsl],
                                     start=(ct == 0), stop=(ct == CT - 1))
                nc.vector.tensor_copy(out=kt_s[0:HD, 1, qsl],
                                      in_=ps_kq2[0:HD, 0:HF])
                # head2 Q lands in parts 64:128; bias-add, then repartition DMA
                q2st = np_pool.tile([P, QC], bf, tag="q2st", name="q2st")
                nc.vector.tensor_scalar_add(out=q2st[HD:P, 0:HF],
                                            in0=ps_kq2[HD:P, 0:HF],
                                            scalar1=bq_s[HD:P, 1:2])
                nc.sync.dma_start(out=qt_s[0:HD, 1, qsl], in_=q2st[HD:P, 0:HF])
            return (6 * HF, f)

        def g_v(kt):
            def f():
                ps_v = pmm.tile([P, QC], f32, tag="mm", name="ps_v")
                for ct in range(CT):
                    nc.tensor.matmul(ps_v[:, 0:HPC * HD],
                                     lhsT=xT_s[:, ct, kt * P:(kt + 1) * P],
                                     rhs=wv_s[:, ct, :],
                                     start=(ct == 0), stop=(ct == CT - 1))
                nc.vector.tensor_copy(
                    out=v_s[:, kt, :, 0:HD],
                    in_=ps_v[:, 0:HPC * HD].rearrange("p (h d) -> p h d", h=HPC))
            return (6 * HPC * HD, f)

        def qkv_items(c, split_late, coarse=False):
            # split_late: K-tiles/V per-ktile, paced into chunk c's own slots
            # (each is only needed at its kt slot). coarse: big groups that
            # pipeline with the staggered initial DMAs (chunk 0 only).
            if coarse:
                items = [g_kk(c, 4 * c, 4 * c + 4), g_qq(c, 0), g_qq(c, 1),
                         g_kq2(c, 0), g_kq2(c, 1)]
                items += [g_v(4 * c + i) for i in range(4)]
                return items, []
            items = [g_qq(c, 0), g_kq2(c, 0), g_qq(c, 1), g_kq2(c, 1)]
            late = []
            for i in range(4):
                if split_late:
                    late.append(g_kk(c, 4 * c + i, 4 * c + i + 1) + (i, True))
                    late.append(g_v(4 * c + i) + (i, False))
                else:
                    items.append(g_kk(c, 4 * c + i, 4 * c + i + 1))
                    items.append(g_v(4 * c + i))
            return items, late

        def emit_scores_exp(c, kt, pt_t):
            # scores (transposed S^T[k,q]) for all 3 heads in one psum tile,
            # exp'd in one ACT instruction per 256-col half into pt_t (SBUF).
            qs = c * QC
            scores_cols = 0
            exp_ns = 0.0
            for half in range(2):
                hbase = half * HF
                off = max(0, kt * P - qs - hbase)
                if off >= HF:
                    continue
                n = HF - off
                ps = ps3.tile([P, HPC, HF], f32, tag="ss", name="ps")
                for h in range(HPC):
                    nc.tensor.matmul(
                        ps[:, h, 0:n],
                        lhsT=kt_s[hsl[h], hslot[h], kt * P:(kt + 1) * P],
                        rhs=qt_s[hsl[h], hslot[h],
                                 qs + hbase + off:qs + hbase + HF],
                        start=True, stop=True)
                if kt * P >= qs:
                    o = kt * P - qs - hbase  # diag block is inside one half
                    if 0 <= o < HF:
                        # additive -1e9 on the k>q triangle, pre-exp in PSUM
                        # (keeps the mask off the exp->AV critical tail)
                        for h in range(HPC):
                            nc.gpsimd.tensor_add(out=ps[:, h, o - off:o - off + P],
                                                 in0=ps[:, h, o - off:o - off + P],
                                                 in1=mask_s)
                nc.scalar.activation(out=pt_t[:, :, hbase + off:hbase + HF],
                                     in_=ps[:, :, 0:n], func=Exp, scale=0.125)
                scores_cols += 3 * n
                exp_ns += 3 * n * 0.833 + 370
            return scores_cols, exp_ns

        # AV accumulator: 12 chains (4 tiles x 3 heads) packed 7-per-bank
        # (73-f32 stride keeps each 65-f32 chain inside one 2KB bank) in one
        # 2-bank PSUM tile; zeroed by memset, accumulated start=False
        def av_slot(pv, i, h):
            fl = 3 * i + h
            o = (fl % 7) * (HD + 9)
            return pv[:, fl // 7, o:o + HD + 1]

        def emit_avs(c, kt, pv, pts, st_by_tile):
            # kt's AV contribution for every tile of the chunk with j >= kt;
            # chains that hit their diagonal finish (stop) and normalize.
            av_cols = 0
            for i in range(4):
                j = 4 * c + i
                if j < kt:
                    continue
                for h in range(HPC):
                    nc.tensor.matmul(av_slot(pv, i, h),
                                     lhsT=pts[kt][:, h, i * P:(i + 1) * P],
                                     rhs=v_s[:, kt, h, :],
                                     start=False, stop=(kt == j),
                                     skip_group_check=True)
                av_cols += HPC * (HD + 1)
                if kt == j:  # normalize tile j now (DVE; frees pav by chunk end)
                    st = st_by_tile[i] = {}
                    st['ao01'] = ao_pool.tile([P, 2, HD], bf, tag="ao01",
                                              name="ao01")
                    st['ao2'] = ao_pool.tile([P, P], bf, tag="ao2", name="ao2")
                    if c + 1 < NQC:  # padded: DMA-transpose reads all 128 cols
                        nc.gpsimd.memset(st['ao2'][:, HD:P], 0.0)
                    for h in range(HPC):
                        sl = av_slot(pv, i, h)
                        recip = np_pool.tile([P, 1], f32, tag="recip",
                                             name="recip")
                        nc.vector.reciprocal(out=recip, in_=sl[:, HD:HD + 1])
                        dst = (st['ao01'][:, h, :] if h < 2
                               else st['ao2'][:, 0:HD])
                        nc.vector.tensor_scalar_mul(out=dst, in0=sl[:, 0:HD],
                                                    scalar1=recip[:, 0:1])
            return av_cols

        def tail_items(c, j, st, blockq, tick, tail=False):
            # transpose (then self-enqueued projection) for q-tile j;
            # deferrable PE fill work. proj re-queues at the back so other
            # items run between transp's DVE copy and proj's read of it.
            def transp():
                aT01 = at_pool.tile([P, P], bf, tag="aT01", name="aT01")
                aT2 = at_pool.tile([P, P], bf, tag="aT2", name="aT2")
                if c + 1 < NQC:
                    # deferred tiles: transpose on the DMA crossbar (no PE
                    # columns, no PSUM, no DVE copies); latency hidden by
                    # the +2-slot proj delay
                    nc.sync.dma_start_transpose(
                        out=aT01, in_=st['ao01'].rearrange("p h d -> p (h d)"))
                    nc.sync.dma_start_transpose(out=aT2, in_=st['ao2'])
                else:
                    psT1 = pmm.tile([P, P], bf, tag="mm", name="psT1")
                    nc.tensor.transpose(
                        out=psT1, in_=st['ao01'].rearrange("p h d -> p (h d)"),
                        identity=ident_s)
                    nc.vector.tensor_copy(out=aT01, in_=psT1)
                    psT2 = pmm.tile([P, P], bf, tag="mm", name="psT2")
                    nc.tensor.transpose(out=psT2[0:HD, :],
                                        in_=st['ao2'][:, 0:HD],
                                        identity=ident_s)
                    nc.vector.tensor_copy(out=aT2[0:HD, :], in_=psT2[0:HD, :])
                st['aT01'], st['aT2'] = aT01, aT2[0:HD, :]
                st['ob'] = outs_pool.tile([P, D], bf, tag="ob", name="ob")
                lag = 2 if c + 1 < NQC else 1
                blockq.append((3 * 512, proj_half(0, 512), tick[0] + lag, BIGT))
                blockq.append((3 * 256, proj_half(512, 256), tick[0] + lag, BIGT))

            def proj_half(e0, en):
                def f():
                    ob = st['ob']
                    pp = pmm.tile([P, QC], f32, tag="mm", name="pp")
                    nc.tensor.matmul(pp[:, 0:en], lhsT=st['aT01'],
                                     rhs=wpa_s[:, e0:e0 + en],
                                     start=True, stop=False)
                    nc.tensor.matmul(pp[:, 0:en], lhsT=st['aT2'][0:HD, :],
                                     rhs=wpb_s[:, e0:e0 + en],
                                     start=False, stop=True)
                    if tail:  # final tile: DVE copy + split DMA, lean tail
                        for q0 in range(e0, e0 + en, 256):
                            qn = min(256, e0 + en - q0)
                            nc.vector.tensor_copy(out=ob[:, q0:q0 + qn],
                                                  in_=pp[:, q0 - e0:q0 - e0 + qn])
                            nc.sync.dma_start(
                                out=out_p[j * P:(j + 1) * P, q0:q0 + qn],
                                in_=ob[:, q0:q0 + qn])
                    else:
                        nc.gpsimd.tensor_copy(out=ob[:, e0:e0 + en],
                                              in_=pp[:, 0:en])
                        nc.sync.dma_start(
                            out=out_p[j * P:(j + 1) * P, e0:e0 + en],
                            in_=ob[:, e0:e0 + en])
                return f

            return [(2 * P, transp, tick[0] + 1, BIGT)]

        # ---- main pipeline with deficit-paced fill work ----
        import os
        BIGT = 10 ** 9
        tick = [0]
        _cpn = os.environ.get("K_COLS_PER_NS", "1.9")
        if "," in _cpn:
            CPN_BY_CHUNK = [float(x) for x in _cpn.split(",")]
        else:
            CPN_BY_CHUNK = [float(_cpn)] * 4
        LATE_BONUS = float(os.environ.get("K_LATE_BONUS", "0.8"))
        AVLAG = int(os.environ.get("K_AVLAG", "2"))
        blockq = deque()

        # chunk 0: minimal head (k-tile 0 + q halves), rest paced in-loop
        for _, f in (g_kk(0, 0, 1), g_qq(0, 0), g_kq2(0, 0),
                     g_qq(0, 1), g_kq2(0, 1)):
            f()
        qkv_late = [g_kk(0, 1, 2) + (1, True), g_v(0) + (0, False),
                    g_kk(0, 2, 3) + (2, True), g_v(1) + (1, False),
                    g_kk(0, 3, 4) + (3, True), g_v(2) + (2, False),
                    g_v(3) + (3, False)]
        BIG = 10 ** 9
        for c in range(NQC):
            nkt = 4 * (c + 1)
            T0 = tick[0]
            for cols, f, rel, is_kk in reversed(qkv_late):
                # hard deadline: kk before its scores slot, v before its AVs
                due = T0 + rel + (1 if is_kk else AVLAG)
                blockq.appendleft((cols, f, 0, due))
            if c + 1 < NQC:
                qkv_due, qkv_late = qkv_items(c + 1, c + 1 == NQC - 1)
            else:
                qkv_due, qkv_late = [], []
            qkv_done = 0
            deficit = 0.0
            pts = []
            st_by_tile = {}
            pv = pav.tile([P, 2, QC], f32, tag="av", name="pv")
            nc.gpsimd.memset(pv, 0.0)
            def fin_tile(kt2):
                # tile kt2's chains just finished (avs emitted): queue tail
                if kt2 >= 4 * c:
                    i = kt2 - 4 * c
                    blockq.extend(tail_items(c, kt2, st_by_tile[i], blockq, tick,
                                             tail=(c == NQC - 1 and i == 3)))

            def pops(limit=None):
                nonlocal deficit, qkv_done
                want = (len(qkv_due) * (kt + 1)) // max(1, nkt - 1)
                while qkv_done < min(want, len(qkv_due)):
                    cols, f = qkv_due[qkv_done]
                    f()
                    qkv_done += 1
                    deficit -= cols
                n = 0
                held = []
                while blockq and deficit > 0 and (limit is None or n < limit):
                    item = blockq.popleft()
                    if item[2] > tick[0]:
                        held.append(item)
                        continue
                    item[1]()
                    deficit -= item[0]
                    n += 1
                for it in reversed(held):
                    blockq.appendleft(it)

            def force_due():
                # emit everything whose deadline has arrived, deficit or not
                nonlocal deficit
                due = [it for it in blockq if it[3] <= tick[0]]
                if due:
                    rest = [it for it in blockq if it[3] > tick[0]]
                    blockq.clear()
                    blockq.extend(rest)
                    for it in due:
                        it[1]()
                        deficit -= it[0]

            for kt in range(nkt):
                tick[0] += 1
                force_due()
                pt_t = ptp.tile([P, HPC, QC], bf, tag="pt", name="pt")
                pts.append(pt_t)
                sc, ens = emit_scores_exp(c, kt, pt_t)
                rate = CPN_BY_CHUNK[c]
                if c == NQC - 1 and kt >= nkt - 6:
                    rate = rate + LATE_BONUS
                deficit += ens * rate - sc
                if kt >= AVLAG:  # AVs run AVLAG exps behind: no ACT waits
                    deficit -= emit_avs(c, kt - AVLAG, pv, pts, st_by_tile)
                    fin_tile(kt - AVLAG)
                pops()
            for kt2 in range(nkt - AVLAG, nkt):  # drain AV lag before next chunk
                tick[0] += 1
                force_due()
                emit_avs(c, kt2, pv, pts, st_by_tile)
                fin_tile(kt2)
                pops(limit=2)
            while qkv_done < len(qkv_due):
                cols, f = qkv_due[qkv_done]
                f()
                qkv_done += 1

        while blockq:
            tick[0] += 1
            item = blockq.popleft()
            item[1]()

    nc.compile()
    return nc


def _prep_inputs(x, W_qkv, b_qkv, W_proj):
    """Build the 8 per-core input maps (all bf16 except biases)."""
    in_maps = []
    for cid in range(NCORES):
        b, g = divmod(cid, 4)
        hs = [g * HPC + i for i in range(HPC)]  # global head ids

        def wslice(kind, h):  # kind 0=q 1=k 2=v
            return W_qkv[:, kind * D + h * HD:(kind * D + (h + 1) * HD)]

        xT = np.ascontiguousarray(x[b].T).astype(_BF)

        w_qk = np.zeros((D, 3, P), dtype=np.float32)
        w_qk[:, 0, 0:HD] = wslice(1, hs[0])
        w_qk[:, 0, HD:P] = wslice(1, hs[1])
        w_qk[:, 1, 0:HD] = wslice(0, hs[0])
        w_qk[:, 1, HD:P] = wslice(0, hs[1])
        w_qk[:, 2, 0:HD] = wslice(1, hs[2])
        w_qk[:, 2, HD:P] = wslice(0, hs[2])

        w_v = np.concatenate([wslice(2, h) for h in hs], axis=1)

        bq = np.zeros((P, 2), dtype=np.float32)
        bq[0:HD, 0] = b_qkv[hs[0] * HD:(hs[0] + 1) * HD]
        bq[HD:P, 0] = b_qkv[hs[1] * HD:(hs[1] + 1) * HD]
        bq[HD:P, 1] = b_qkv[hs[2] * HD:(hs[2] + 1) * HD]

        w_p = np.concatenate([W_proj[h * HD:(h + 1) * HD, :] for h in hs], axis=0)

        # additive pre-exp mask: 0 where q >= k (valid), -1e9 on the triangle
        mask = (np.triu(np.ones((P, P), dtype=np.float32)) - 1.0) * 1e9
        ident = np.eye(P, dtype=np.float32)

        in_maps.append({
            "xT": xT,
            "w_qk": w_qk.astype(_BF),
            "w_v": w_v.astype(_BF),
            "bq": bq,
            "w_p": w_p.astype(_BF),
            "mask": mask.astype(_BF),
            "ident": ident.astype(_BF),
        })
    return in_maps


def _run(inputs, trace=False):
    from concourse.bass_utils import run_bass_kernel_spmd

    x = np.asarray(inputs["x"], dtype=np.float32)
    W_qkv = np.asarray(inputs["W_qkv"], dtype=np.float32)
    b_qkv = np.asarray(inputs["b_qkv"], dtype=np.float32)
    W_proj = np.asarray(inputs["W_proj"], dtype=np.float32)
    b_proj = np.asarray(inputs["b_proj"], dtype=np.float32)

    if "nc" not in _cache:
        _cache["nc"] = _build_nc()
    nc = _cache["nc"]

    in_maps = _prep_inputs(x, W_qkv, b_qkv, W_proj)
    res = run_bass_kernel_spmd(nc, in_maps, core_ids=list(range(NCORES)),
                               trace=trace)

    host_bias = b_proj + b_qkv[2 * D:3 * D] @ W_proj  # b_v folded through proj
    B = x.shape[0]
    out = np.zeros((B, S, D), dtype=np.float32)
    for cid in range(NCORES):
        b = cid // 4
        out[b] += res.results[cid]["out_p"].astype(np.float32)
    out += host_bias
    return out, res


def kernel(x, W_qkv, b_qkv, W_proj, b_proj):
    out, _ = _run({"x": x, "W_qkv": W_qkv, "b_qkv": b_qkv,
                   "W_proj": W_proj, "b_proj": b_proj})
    return out
